# revision 38
# baseline (speedup 1.0000x reference)
"""Trainium2 Bass kernel for nn_DenseBlockEnd (ragged masked residual-add + relu).

Op: out[g] = relu(features[g] + residuals[0,g] + residuals[1,g]) for rows < M_g,
    zeros for rows >= M_g  (M_g = mol_slice[g, 0]).

Strategy (8 NeuronCores, SPMD via run_bass_kernel_spmd):
- Shard the batch (B=256 graphs) across 8 cores, snake-draft balanced on total
  valid rows so per-core HBM traffic is equal.
- All raggedness is resolved HOST-side: each core receives one densely packed
  bf16 stream (only the valid rows, concatenated, padded with zeros to a
  common length). The device kernel is a uniform streaming elementwise
  kernel: fully-contiguous tile loads, 3-way add + relu on DVE, contiguous
  stores. bf16 halves HBM traffic vs f32; the rel-err budget (2e-2) dwarfs
  bf16 rounding (~5e-3).
- Compute: two contiguous tensor_tensor adds + tensor_scalar_max on DVE
  (measured: contiguous tensor_tensor ~2x faster than tensor_reduce over a
  size-3 axis, strided or not). Identical instruction stream on every core.
- Rows >= M_g are never touched: ExternalOutput buffers are zero-initialized
  by the runtime, and the host unpack only reads back valid rows.
"""

import sys

sys.path.insert(0, "/opt/trn_rl_repo")

import ml_dtypes
import numpy as np

import concourse.bass as bass
import concourse.mybir as mybir
from concourse.alu_op_type import AluOpType
import bass_rust
import concourse.tile as tile
from concourse.bass_utils import run_bass_kernel_spmd
from concourse.tile import TileContext
from concourse.vector_clock import ScopedClock

B, A, F = 256, 128, 1024
N_CORES = 8
G_PER_CORE = B // N_CORES
BF16 = ml_dtypes.bfloat16


def _drain_and_barrier_split(self, tick_clock, wait_clock):
    # This container's walrus rejects instructions carrying more than one sem
    # wait ("Too many sync wait commands" at the kernel-tail Drain). Collect
    # the final waits on a probe instruction and emit them as single-wait
    # NOPs on the sync engine before a clean drain.
    probe = mybir.InstNoOp(
        name=self.nc.get_next_instruction_name(), engine=mybir.EngineType.SP
    )
    wait_clock.add_sem_waits(probe, ScopedClock({None: tick_clock.global_clock}))
    waits = list(probe.sync_info.on_wait) if probe.sync_info else []
    for w in waits:
        ins = self.nc.sync.nop(nofuse=True)
        si = ins.ins.sync_info
        if si is None:
            ins.ins.sync_info = mybir.SyncInfo(on_wait=[w], on_update=[])
        else:
            si.on_wait.append(w)
    self.nc.sync.drain()
    self.nc.all_engine_barrier()
    assert self.sems is not None
    popped = self.nc._tile_sem_poison_stack.pop()
    assert popped is self._sem_poison
    self.nc.clear_and_free_semaphores(list(self.sems.allocated().values()))
    if not getattr(self, "_skip_final_barrier", False):
        self.nc.all_engine_barrier()


tile.TileContext._drain_and_barrier = _drain_and_barrier_split

_orig_lower_ordered_insts = tile.TileContext._lower_ordered_insts


def _lower_with_wait_split(self, ordered):
    # Same walrus limitation as above, applied to every scheduled
    # instruction: hoist all but one sem wait onto single-wait NOPs emitted
    # just before the instruction on the same engine.
    for insts in ordered.values():
        if not any(
            i.sync_info is not None and len(i.sync_info.on_wait) > 1 for i in insts
        ):
            continue
        new_list = []
        for inst in insts:
            si = inst.sync_info
            if si is not None and len(si.on_wait) > 1:
                for w in si.on_wait[1:]:
                    new_list.append(
                        mybir.InstNoOp(
                            name=self.nc.get_next_instruction_name(),
                            engine=inst.engine,
                            sync_info=mybir.SyncInfo(on_wait=[w], on_update=[]),
                            bass_nofuse=True,
                        )
                    )
                si.on_wait = si.on_wait[:1]
            new_list.append(inst)
        insts[:] = new_list
    return _orig_lower_ordered_insts(self, ordered)


tile.TileContext._lower_ordered_insts = _lower_with_wait_split


def _assign_graphs(m: np.ndarray) -> list[list[int]]:
    """Snake-draft 256 graphs into 8 groups of 32, balancing sum(M)."""
    order = np.argsort(-m, kind="stable")
    groups: list[list[int]] = [[] for _ in range(N_CORES)]
    for rnd in range(G_PER_CORE):
        idxs = order[rnd * N_CORES : (rnd + 1) * N_CORES]
        seq = range(N_CORES) if rnd % 2 == 0 else range(N_CORES - 1, -1, -1)
        for c, g in zip(seq, idxs):
            groups[c].append(int(g))
    return groups


def _plan(rmax: int, w_target: int = 1408) -> tuple[tuple[int, ...], int]:
    """Tile geometry for a packed stream of rmax*F elems.

    Returns (widths, L): per-tile per-partition widths and the padded stream
    length L = 128 * sum(widths). Mostly uniform W tiles, with half-width
    tiles at the head (faster pipeline ramp: the first tiles complete
    early so DVE starts sooner) and at the tail (shorter last-tile chain).
    """
    lmax = rmax * F
    total = -(-lmax // (128 * 32)) * 32  # per-partition elems, mult of 32
    w = w_target
    h = w // 2
    head = [h, h]
    tail = [h, h, h // 2, 192, 160]
    n_full = max(0, -(-(total - sum(head) - sum(tail)) // w))
    widths = head + [w] * n_full + tail
    # trim overshoot from the last full tiles to keep padding minimal
    slack = (sum(widths) - total) // 32 * 32
    for j in range(len(head) + n_full - 1, -1, -1):
        cut = min(slack, widths[j] - 160)
        if cut > 0:
            widths[j] -= cut
            slack -= cut
        if slack <= 0:
            break
    return tuple(widths), 128 * sum(widths)


def _build_program(widths: tuple[int, ...]):
    L = 128 * sum(widths)
    wmax = max(widths)
    n_t = len(widths)
    nc = bass.Bass()
    x_ext = nc.dram_tensor("x", [3 * L], mybir.dt.bfloat16, kind="ExternalInput")
    o_ext = nc.dram_tensor("o", [L], mybir.dt.bfloat16, kind="ExternalOutput")

    # Loads are all enqueued up front, so a load waiting on buffer reuse
    # would head-of-line block its HWDGE queue — except the LAST load on
    # each queue (nothing queues behind it). So n_t - 2 buffers suffice
    # when all n_t don't fit in SBUF.
    for bufs, obufs in ((n_t, n_t), (n_t - 2, 6)):
        if bufs * 6 * wmax + (obufs + 2) * 2 * wmax <= 195_000:
            break
    else:
        raise AssertionError("SBUF budget exceeded")

    with TileContext(nc) as tc:
        with (
            nc.allow_low_precision(reason="bf16 3-way add; tol 2e-2 >> bf16 ulp"),
            tc.tile_pool(name="p", bufs=bufs) as pool,
            tc.tile_pool(name="pt", bufs=2) as tpool,
            tc.tile_pool(name="po", bufs=obufs) as opool,
        ):
            # Pass 1: ALL load dma_starts up front, whole tiles alternating
            # between the two HWDGE queues. Emitting loads before any ACT
            # compute keeps the scalar engine's queue fed without
            # head-of-line blocking behind Relu instructions.
            # (Half-partition splits across both queues were tried and
            # halved per-packet DMA throughput: two queues streaming
            # adjacent HBM regions concurrently conflict; alternating
            # whole tiles keeps the queues in distant address regions.)
            tiles = []
            off = 0
            for i, w in enumerate(widths):
                t = pool.tile([128, 3 * w], mybir.dt.bfloat16, tag="t")
                ld = nc.sync if i % 2 == 0 else nc.scalar
                ld.dma_start(
                    out=t[:],
                    in_=x_ext[3 * off : 3 * (off + 128 * w)].rearrange(
                        "(p q) -> p q", p=128
                    ),
                )
                tiles.append((t, off))
                off += 128 * w
            # Pass 2: per-tile compute + store. Adds on DVE; Relu on ACT
            # (its loads are already enqueued, so no queue blocking; this
            # splits the elementwise work across both engines).
            for i, w in enumerate(widths):
                t, off = tiles[i]
                to = opool.tile([128, w], mybir.dt.bfloat16, tag="to")
                tmp = tpool.tile([128, w], mybir.dt.bfloat16, tag="tmp")
                nc.vector.tensor_tensor(
                    out=tmp[:], in0=t[:, 0:w], in1=t[:, w : 2 * w], op=AluOpType.add
                )
                nc.vector.tensor_tensor(
                    out=to[:],
                    in0=tmp[:],
                    in1=t[:, 2 * w : 3 * w],
                    op=AluOpType.add,
                )
                # relu on DVE: ACT's Activation is ~3x slower per element and
                # serializes the pipeline tail; gpsimd tensor ops are ~40x
                # slower (ucode-emulated)
                nc.vector.tensor_scalar_max(out=to[:], in0=to[:], scalar1=0.0)
                # last stores on the HWDGE queues: all loads are enqueued
                # ahead of them (FIFO) and finished by the time these fire,
                # so they can't block a load — and SWDGE's final drain
                # starts earlier and overlaps the HWDGE tail stores
                if i >= len(widths) - 4:
                    st = nc.sync if i % 2 == 0 else nc.scalar
                else:
                    st = nc.gpsimd
                st.dma_start(
                    out=o_ext[off : off + 128 * w].rearrange("(p w) -> p w", p=128),
                    in_=to[:],
                )
        tc._skip_final_barrier = True
    _exempt_from_entry_barrier(nc)
    return nc


def _exempt_from_entry_barrier(nc):
    """Let the SP engine skip (and not even Drain at) the kernel-entry barrier.

    The preamble barrier only guards the Pool-engine const-AP memsets (which
    SP never reads: it only issues load DMAs) while absorbing engine start
    skew. Removing its Drain + wait lets its first load DMAs start
    immediately. The Activation engine stays in the barrier: its Relu reads
    the const-AP bias the Pool memsets initialize. The barrier protocol is
    self-resetting, so only the entry barrier leader's counts change
    (4 -> 3).
    """
    exempt = (mybir.EngineType.SP,)
    f0 = nc.m.functions[0]
    bb0 = f0.blocks[0]
    pool = mybir.EngineType.Pool
    arrive_id = None
    # pass 1: find the barrier arrive sem id from any exempt engine's Drain
    for ins in bb0.instructions:
        if ins.engine in exempt and ins.sync_info is not None:
            if ins.opcode == "Drain" and ins.sync_info.on_update:
                arrive_id = ins.sync_info.on_update[0].id
                break
    if arrive_id is None:
        return
    # pass 2: drop the exempt engines' arrive Drain and release wait (first
    # EventSemaphore per engine: that's the entry-barrier release wait)
    drop = []
    seen_ev = set()
    for ins in bb0.instructions:
        if ins.engine not in exempt or ins.sync_info is None:
            continue
        if ins.opcode == "Drain" and ins.sync_info.on_update:
            drop.append(ins)
        elif ins.opcode == "EventSemaphore" and ins.engine not in seen_ev:
            seen_ev.add(ins.engine)
            drop.append(ins)
    if len(drop) != 2 * len(exempt):
        return
    for ins in drop:
        bb0.instructions.remove(ins)
    n = 4 - len(exempt)
    for ins in bb0.instructions:
        if ins.engine != pool or ins.opcode != "EventSemaphore" or ins.sync_info is None:
            continue
        si = ins.sync_info
        for w in si.on_wait:
            if w.id == arrive_id and w.wait_value == 4:
                w.wait_value = n
        for u in si.on_update:
            if u.update_value == 4:
                u.update_value = n


_PROGRAM_CACHE: dict = {}


def _prepare(features, residuals, mol_slice):
    features = np.asarray(features, dtype=np.float32)
    residuals = np.asarray(residuals, dtype=np.float32)
    m = np.asarray(mol_slice)[:, 0].astype(np.int64)
    assert features.shape == (B, A, F) and residuals.shape == (2, B, A, F)

    groups = _assign_graphs(m)
    rmax = max(int(m[np.array(g)].sum()) for g in groups)
    widths, L = _plan(rmax)

    nc = _PROGRAM_CACHE.get(widths)
    if nc is None:
        nc = _build_program(widths)
        _PROGRAM_CACHE[widths] = nc

    streams = (features, residuals[0], residuals[1])
    in_maps = []
    for c in range(N_CORES):
        # planar packed stream S[3, L] (f32) of this core's valid rows
        S = np.zeros((3, L), dtype=np.float32)
        pos = 0
        for g in groups[c]:
            mg = int(m[g])
            n = mg * F
            for s in range(3):
                S[s, pos : pos + n] = streams[s][g, :mg].reshape(-1)
            pos += n
        # per-tile on-device layout: [tile i][partition p][stream s][w_i]
        x = np.empty(3 * L, dtype=BF16)
        off = 0
        for w in widths:
            cw = 128 * w
            x[3 * off : 3 * (off + cw)].reshape(128, 3, w)[...] = S[
                :, off : off + cw
            ].reshape(3, 128, w).transpose(1, 0, 2)
            off += cw
        in_maps.append({"x": x})
    return nc, in_maps, groups, m


def _unpack(results, groups, m):
    out = np.zeros((B, A, F), dtype=np.float32)
    for c in range(N_CORES):
        o = results[c]["o"]
        pos = 0
        for g in groups[c]:
            mg = int(m[g])
            n = mg * F
            out[g, :mg] = o[pos : pos + n].reshape(mg, F)
            pos += n
    return out


def kernel(features, residuals, mol_slice):
    nc, in_maps, groups, m = _prepare(features, residuals, mol_slice)
    res = run_bass_kernel_spmd(nc, in_maps, list(range(N_CORES)))
    return _unpack(res.results, groups, m)


# revision 40
# speedup vs baseline: 1.0174x; 1.0174x over previous
"""Trainium2 Bass kernel for nn_DenseBlockEnd (ragged masked residual-add + relu).

Op: out[g] = relu(features[g] + residuals[0,g] + residuals[1,g]) for rows < M_g,
    zeros for rows >= M_g  (M_g = mol_slice[g, 0]).

Strategy (8 NeuronCores, SPMD via run_bass_kernel_spmd):
- Shard the batch (B=256 graphs) across 8 cores, snake-draft balanced on total
  valid rows so per-core HBM traffic is equal.
- All raggedness is resolved HOST-side: each core receives one densely packed
  bf16 stream (only the valid rows, concatenated, padded with zeros to a
  common length). The device kernel is a uniform streaming elementwise
  kernel: fully-contiguous tile loads, 3-way add + relu on DVE, contiguous
  stores. bf16 halves HBM traffic vs f32; the rel-err budget (2e-2) dwarfs
  bf16 rounding (~5e-3).
- Compute: two contiguous tensor_tensor adds + tensor_scalar_max on DVE
  (measured: contiguous tensor_tensor ~2x faster than tensor_reduce over a
  size-3 axis, strided or not). Identical instruction stream on every core.
- Rows >= M_g are never touched: ExternalOutput buffers are zero-initialized
  by the runtime, and the host unpack only reads back valid rows.
"""

import sys

sys.path.insert(0, "/opt/trn_rl_repo")

import ml_dtypes
import numpy as np

import concourse.bass as bass
import concourse.mybir as mybir
from concourse.alu_op_type import AluOpType
import bass_rust
import concourse.tile as tile
from concourse.bass_utils import run_bass_kernel_spmd
from concourse.tile import TileContext
from concourse.vector_clock import ScopedClock

B, A, F = 256, 128, 1024
N_CORES = 8
G_PER_CORE = B // N_CORES
BF16 = ml_dtypes.bfloat16


def _drain_and_barrier_split(self, tick_clock, wait_clock):
    # This container's walrus rejects instructions carrying more than one sem
    # wait ("Too many sync wait commands" at the kernel-tail Drain). Collect
    # the final waits on a probe instruction and emit them as single-wait
    # NOPs on the sync engine before a clean drain.
    probe = mybir.InstNoOp(
        name=self.nc.get_next_instruction_name(), engine=mybir.EngineType.SP
    )
    wait_clock.add_sem_waits(probe, ScopedClock({None: tick_clock.global_clock}))
    waits = list(probe.sync_info.on_wait) if probe.sync_info else []
    for w in waits:
        ins = self.nc.sync.nop(nofuse=True)
        si = ins.ins.sync_info
        if si is None:
            ins.ins.sync_info = mybir.SyncInfo(on_wait=[w], on_update=[])
        else:
            si.on_wait.append(w)
    self.nc.sync.drain()
    self.nc.all_engine_barrier()
    assert self.sems is not None
    popped = self.nc._tile_sem_poison_stack.pop()
    assert popped is self._sem_poison
    self.nc.clear_and_free_semaphores(list(self.sems.allocated().values()))
    if not getattr(self, "_skip_final_barrier", False):
        self.nc.all_engine_barrier()


tile.TileContext._drain_and_barrier = _drain_and_barrier_split

_orig_lower_ordered_insts = tile.TileContext._lower_ordered_insts


def _lower_with_wait_split(self, ordered):
    # Same walrus limitation as above, applied to every scheduled
    # instruction: hoist all but one sem wait onto single-wait NOPs emitted
    # just before the instruction on the same engine.
    for insts in ordered.values():
        if not any(
            i.sync_info is not None and len(i.sync_info.on_wait) > 1 for i in insts
        ):
            continue
        new_list = []
        for inst in insts:
            si = inst.sync_info
            if si is not None and len(si.on_wait) > 1:
                for w in si.on_wait[1:]:
                    new_list.append(
                        mybir.InstNoOp(
                            name=self.nc.get_next_instruction_name(),
                            engine=inst.engine,
                            sync_info=mybir.SyncInfo(on_wait=[w], on_update=[]),
                            bass_nofuse=True,
                        )
                    )
                si.on_wait = si.on_wait[:1]
            new_list.append(inst)
        insts[:] = new_list
    return _orig_lower_ordered_insts(self, ordered)


tile.TileContext._lower_ordered_insts = _lower_with_wait_split


def _assign_graphs(m: np.ndarray) -> list[list[int]]:
    """Snake-draft 256 graphs into 8 groups of 32, balancing sum(M)."""
    order = np.argsort(-m, kind="stable")
    groups: list[list[int]] = [[] for _ in range(N_CORES)]
    for rnd in range(G_PER_CORE):
        idxs = order[rnd * N_CORES : (rnd + 1) * N_CORES]
        seq = range(N_CORES) if rnd % 2 == 0 else range(N_CORES - 1, -1, -1)
        for c, g in zip(seq, idxs):
            groups[c].append(int(g))
    return groups


def _plan(rmax: int, w_target: int = 1792) -> tuple[tuple[int, ...], int]:
    """Tile geometry for a packed stream of rmax*F elems.

    Returns (widths, L): per-tile per-partition widths and the padded stream
    length L = 128 * sum(widths). Mostly uniform W tiles, with half-width
    tiles at the head (faster pipeline ramp: the first tiles complete
    early so DVE starts sooner) and at the tail (shorter last-tile chain).
    """
    lmax = rmax * F
    total = -(-lmax // (128 * 32)) * 32  # per-partition elems, mult of 32
    w = w_target
    h = w // 2
    head = [h, h]
    tail = [h, h, h // 2, h // 2]
    n_full = max(0, -(-(total - sum(head) - sum(tail)) // w))
    widths = head + [w] * n_full + tail
    # trim overshoot from the last full tiles to keep padding minimal
    slack = (sum(widths) - total) // 32 * 32
    for j in range(len(head) + n_full - 1, -1, -1):
        cut = min(slack, widths[j] - 160)
        if cut > 0:
            widths[j] -= cut
            slack -= cut
        if slack <= 0:
            break
    return tuple(widths), 128 * sum(widths)


def _build_program(widths: tuple[int, ...]):
    L = 128 * sum(widths)
    wmax = max(widths)
    n_t = len(widths)
    nc = bass.Bass()
    x_ext = nc.dram_tensor("x", [3 * L], mybir.dt.bfloat16, kind="ExternalInput")
    o_ext = nc.dram_tensor("o", [L], mybir.dt.bfloat16, kind="ExternalOutput")

    # Loads are all enqueued up front, so a load waiting on buffer reuse
    # would head-of-line block its HWDGE queue — except the LAST load on
    # each queue (nothing queues behind it). So n_t - 2 buffers suffice
    # when all n_t don't fit in SBUF.
    for bufs, obufs in ((n_t, n_t), (n_t - 2, 6)):
        if bufs * 6 * wmax + (obufs + 2) * 2 * wmax <= 195_000:
            break
    else:
        raise AssertionError("SBUF budget exceeded")

    with TileContext(nc) as tc:
        with (
            nc.allow_low_precision(reason="bf16 3-way add; tol 2e-2 >> bf16 ulp"),
            tc.tile_pool(name="p", bufs=bufs) as pool,
            tc.tile_pool(name="pt", bufs=2) as tpool,
            tc.tile_pool(name="po", bufs=obufs) as opool,
        ):
            # Pass 1: ALL load dma_starts up front, whole tiles alternating
            # between the two HWDGE queues. Emitting loads before any ACT
            # compute keeps the scalar engine's queue fed without
            # head-of-line blocking behind Relu instructions.
            # (Half-partition splits across both queues were tried and
            # halved per-packet DMA throughput: two queues streaming
            # adjacent HBM regions concurrently conflict; alternating
            # whole tiles keeps the queues in distant address regions.)
            tiles = []
            off = 0
            for i, w in enumerate(widths):
                t = pool.tile([128, 3 * w], mybir.dt.bfloat16, tag="t")
                ld = nc.sync if i % 2 == 0 else nc.scalar
                ld.dma_start(
                    out=t[:],
                    in_=x_ext[3 * off : 3 * (off + 128 * w)].rearrange(
                        "(p q) -> p q", p=128
                    ),
                )
                tiles.append((t, off))
                off += 128 * w
            # Pass 2: per-tile compute + store. Adds on DVE; Relu on ACT
            # (its loads are already enqueued, so no queue blocking; this
            # splits the elementwise work across both engines).
            for i, w in enumerate(widths):
                t, off = tiles[i]
                to = opool.tile([128, w], mybir.dt.bfloat16, tag="to")
                tmp = tpool.tile([128, w], mybir.dt.bfloat16, tag="tmp")
                nc.vector.tensor_tensor(
                    out=tmp[:], in0=t[:, 0:w], in1=t[:, w : 2 * w], op=AluOpType.add
                )
                nc.vector.tensor_tensor(
                    out=to[:],
                    in0=tmp[:],
                    in1=t[:, 2 * w : 3 * w],
                    op=AluOpType.add,
                )
                # relu on DVE: ACT's Activation is ~3x slower per element and
                # serializes the pipeline tail; gpsimd tensor ops are ~40x
                # slower (ucode-emulated)
                nc.vector.tensor_scalar_max(out=to[:], in0=to[:], scalar1=0.0)
                # last stores on the HWDGE queues: all loads are enqueued
                # ahead of them (FIFO) and finished by the time these fire,
                # so they can't block a load — and SWDGE's final drain
                # starts earlier and overlaps the HWDGE tail stores
                if i >= len(widths) - 4:
                    st = nc.sync if i % 2 == 0 else nc.scalar
                else:
                    st = nc.gpsimd
                st.dma_start(
                    out=o_ext[off : off + 128 * w].rearrange("(p w) -> p w", p=128),
                    in_=to[:],
                )
        tc._skip_final_barrier = True
    _exempt_from_entry_barrier(nc)
    return nc


def _exempt_from_entry_barrier(nc):
    """Let the SP engine skip (and not even Drain at) the kernel-entry barrier.

    The preamble barrier only guards the Pool-engine const-AP memsets (which
    SP never reads: it only issues load DMAs) while absorbing engine start
    skew. Removing its Drain + wait lets its first load DMAs start
    immediately. The Activation engine stays in the barrier: its Relu reads
    the const-AP bias the Pool memsets initialize. The barrier protocol is
    self-resetting, so only the entry barrier leader's counts change
    (4 -> 3).
    """
    exempt = (mybir.EngineType.SP,)
    f0 = nc.m.functions[0]
    bb0 = f0.blocks[0]
    pool = mybir.EngineType.Pool
    arrive_id = None
    # pass 1: find the barrier arrive sem id from any exempt engine's Drain
    for ins in bb0.instructions:
        if ins.engine in exempt and ins.sync_info is not None:
            if ins.opcode == "Drain" and ins.sync_info.on_update:
                arrive_id = ins.sync_info.on_update[0].id
                break
    if arrive_id is None:
        return
    # pass 2: drop the exempt engines' arrive Drain and release wait (first
    # EventSemaphore per engine: that's the entry-barrier release wait)
    drop = []
    seen_ev = set()
    for ins in bb0.instructions:
        if ins.engine not in exempt or ins.sync_info is None:
            continue
        if ins.opcode == "Drain" and ins.sync_info.on_update:
            drop.append(ins)
        elif ins.opcode == "EventSemaphore" and ins.engine not in seen_ev:
            seen_ev.add(ins.engine)
            drop.append(ins)
    if len(drop) != 2 * len(exempt):
        return
    for ins in drop:
        bb0.instructions.remove(ins)
    n = 4 - len(exempt)
    for ins in bb0.instructions:
        if ins.engine != pool or ins.opcode != "EventSemaphore" or ins.sync_info is None:
            continue
        si = ins.sync_info
        for w in si.on_wait:
            if w.id == arrive_id and w.wait_value == 4:
                w.wait_value = n
        for u in si.on_update:
            if u.update_value == 4:
                u.update_value = n


_PROGRAM_CACHE: dict = {}


def _prepare(features, residuals, mol_slice):
    features = np.asarray(features, dtype=np.float32)
    residuals = np.asarray(residuals, dtype=np.float32)
    m = np.asarray(mol_slice)[:, 0].astype(np.int64)
    assert features.shape == (B, A, F) and residuals.shape == (2, B, A, F)

    groups = _assign_graphs(m)
    rmax = max(int(m[np.array(g)].sum()) for g in groups)
    widths, L = _plan(rmax)

    nc = _PROGRAM_CACHE.get(widths)
    if nc is None:
        nc = _build_program(widths)
        _PROGRAM_CACHE[widths] = nc

    streams = (features, residuals[0], residuals[1])
    in_maps = []
    for c in range(N_CORES):
        # planar packed stream S[3, L] (f32) of this core's valid rows
        S = np.zeros((3, L), dtype=np.float32)
        pos = 0
        for g in groups[c]:
            mg = int(m[g])
            n = mg * F
            for s in range(3):
                S[s, pos : pos + n] = streams[s][g, :mg].reshape(-1)
            pos += n
        # per-tile on-device layout: [tile i][partition p][stream s][w_i]
        x = np.empty(3 * L, dtype=BF16)
        off = 0
        for w in widths:
            cw = 128 * w
            x[3 * off : 3 * (off + cw)].reshape(128, 3, w)[...] = S[
                :, off : off + cw
            ].reshape(3, 128, w).transpose(1, 0, 2)
            off += cw
        in_maps.append({"x": x})
    return nc, in_maps, groups, m


def _unpack(results, groups, m):
    out = np.zeros((B, A, F), dtype=np.float32)
    for c in range(N_CORES):
        o = results[c]["o"]
        pos = 0
        for g in groups[c]:
            mg = int(m[g])
            n = mg * F
            out[g, :mg] = o[pos : pos + n].reshape(mg, F)
            pos += n
    return out


def kernel(features, residuals, mol_slice):
    nc, in_maps, groups, m = _prepare(features, residuals, mol_slice)
    res = run_bass_kernel_spmd(nc, in_maps, list(range(N_CORES)))
    return _unpack(res.results, groups, m)
